# revision 27
# baseline (speedup 1.0000x reference)
"""Multi-head attention (B=2, S=2048, D=1024, H=16, Dh=64) on 8 Trainium2
NeuronCores via Bass/Tile.

Sharding: data-parallel over the 2 batches x tensor-parallel over head
groups (16 heads -> 4 groups of 4). Core c = 4*b + g handles batch b and
heads 4g..4g+3 with the matching column/row slices of Wq/Wk/Wv/Wo. Each
core returns its partial output projection (bf16); the host sums the 4
partials per batch and adds bo.

v2 vs v1:
  - x is pre-cast to bf16 on host; x^T materialized via DMA XBAR
    transposes (DRAM -> SBUF) instead of PE transpose + DVE evacuation.
  - exp split across engines: ACT does the first AW columns of each
    score tile exactly; DVE does the rest with a Schraudolph-style
    int16 bit-trick (bits = KA*score + KB, bitcast as bf16 ~= exp).
    The softmax denominator uses the same approx values, so the
    common-mode error cancels; residual ~2% rms on half the heads.
  - V bias folded into the V-projection matmul via a K=1 ones-row MM;
    V/QK PSUM evacuations moved to the ACT engine (activation Copy /
    Identity with per-partition scale/bias).
  - weights loaded as fp32 via fast sync DMA and cast to bf16 on DVE
    (frees the slow SWDGE casting queues; x no longer needs them).
  - out partials stored bf16 (halves the output DMA).

Per-core kernel (4 heads = 2 "pairs" of 64-dim heads stacked to fill the
128-partition dim), bf16 matmul datapath with fp32 PSUM accumulation:
  xT   = dma_transpose(x_bf16)          [128 (d), 8, S]
  QT   = Wq_g^T x^T + bq_g              [128 (2 heads x 64), 2 pairs, S]
  KT   = Wk_g^T x^T + bk_g              (same layout)
  V_ext= [(x Wv_g + bv_g) * maskf | maskf]   [s, chunk, 4*(64+1)] bf16
  per pair, per q-tile (512 queries), per key chunk (128 keys):
    scT [128k, 2x512q] = KT_chunk^T @ QT_tile   (2 heads row-tiled, concurrent)
    eT  = exp(SCALE * scT)   (ACT exact on cols<AW, DVE Schraudolph on rest)
    ctx_h[65, 512] += V_ext_chunk^T @ eT_h      (row 64 = softmax denom)
  normalize: den -> 1/x (DVE) -> broadcast (GPSIMD) -> ctxT = ctx*rec (DVE)
  out_partial = ctxT^T @ Wo_g           (PSUM accum over the 2 pairs)
"""

import numpy as np
import ml_dtypes

import concourse.bacc as bacc
import concourse.mybir as mybir
import concourse.tile as tile
from concourse.bass_utils import run_bass_kernel_spmd
from concourse.masks import make_identity

F32 = mybir.dt.float32
BF16 = mybir.dt.bfloat16
I16 = mybir.dt.int16
AF = mybir.ActivationFunctionType
OP = mybir.AluOpType

S = 2048
D = 1024
HPC = 4                  # heads per core
DH = 64
PAIRS = 2                # head pairs per core
P = 128
SC_CHUNKS = S // P       # 16 key chunks
QT_TILES = 4             # q tiles of 512
QW = 512                 # q tile width
ST_TILES = S // P        # 16 s tiles
DCH = D // P             # 8 D chunks
SCALE = 1.0 / np.sqrt(DH)

# exp split: ACT handles et[:, 0:AW] exactly; DVE approximates the rest.
AW = 512
# Schraudolph constants for bf16: bits = KA*score + KB, int16-cast,
# bitcast bf16 ~= exp(SCALE*score). C=5.35 calibrated (maxrel ~3.3%).
KA = float(SCALE * 128.0 / np.log(2.0))
KB = float(16256.0 - 5.35)

N_CORES = 8


def build():
    nc = bacc.Bacc(None, target_bir_lowering=False, num_swdge_queues=4)

    x = nc.dram_tensor("x", [S, D], BF16, kind="ExternalInput")
    wq = nc.dram_tensor("wq", [D, 256], F32, kind="ExternalInput")
    wk = nc.dram_tensor("wk", [D, 256], F32, kind="ExternalInput")
    wv = nc.dram_tensor("wv", [D, 256], F32, kind="ExternalInput")
    wo = nc.dram_tensor("wo", [256, D], F32, kind="ExternalInput")
    bq = nc.dram_tensor("bq", [P, PAIRS], F32, kind="ExternalInput")
    bk = nc.dram_tensor("bk", [P, PAIRS], F32, kind="ExternalInput")
    bv = nc.dram_tensor("bv", [256], F32, kind="ExternalInput")
    maskf = nc.dram_tensor("maskf", [P, SC_CHUNKS], F32, kind="ExternalInput")
    out = nc.dram_tensor("out", [S, D], BF16, kind="ExternalOutput")

    with tile.TileContext(nc) as tc:
        with (
            tc.tile_pool(name="persist", bufs=1) as pp,
            tc.tile_pool(name="xstage", bufs=3) as xs,
            tc.tile_pool(name="wstage", bufs=2) as ws,
            tc.tile_pool(name="expp", bufs=5) as ep,
            tc.tile_pool(name="ostage", bufs=2) as op_,
            tc.tile_pool(name="smalls", bufs=3) as sp,
            tc.tile_pool(name="ps_sc", bufs=2, space="PSUM") as ps_sc,
            tc.tile_pool(name="ps_ctx", bufs=2, space="PSUM") as ps_ctx,
            tc.tile_pool(name="ps_w", bufs=2, space="PSUM") as ps_w,
        ):
            # ---- persistent big tensors ----
            xT = pp.tile([P, DCH, S], BF16)
            QT = pp.tile([P, PAIRS, S], BF16)
            KT = pp.tile([P, PAIRS, S], BF16)
            VE = pp.tile([P, SC_CHUNKS, HPC * (DH + 1)], BF16)
            ctxT = pp.tile([P, PAIRS, S], BF16)
            wq_sb = pp.tile([P, DCH, 256], BF16)
            wk_sb = pp.tile([P, DCH, 256], BF16)
            wv_sb = pp.tile([P, DCH, 256], BF16)
            wo_sb = pp.tile([P, PAIRS, D], BF16)

            # x^T via PE transpose-mode (DMA XBAR transpose is only ~60GB/s
            # and stalls the prefix). 8 transposes batch into one PSUM
            # region, evacuated by a single ACT copy per s-tile.
            ident = pp.tile([P, P], BF16)
            make_identity(nc, ident[:])

            def transpose_st(st):
                xst = xs.tile([P, D], BF16, tag="xst")
                # two half-loads -> two DMA engines per tile (less jitter)
                nc.sync.dma_start(
                    xst[:, 0 : D // 2], x[st * P : (st + 1) * P, 0 : D // 2]
                )
                nc.sync.dma_start(
                    xst[:, D // 2 : D], x[st * P : (st + 1) * P, D // 2 : D]
                )
                pt = ps_sc.tile([P, DCH * P], BF16, tag="sc")
                for dc in range(DCH):
                    nc.tensor.transpose(
                        pt[:, dc * P : (dc + 1) * P],
                        xst[:, dc * P : (dc + 1) * P],
                        ident[:],
                    )
                nc.scalar.activation(
                    xT[:, :, st * P : (st + 1) * P],
                    pt[:].rearrange("p (c n) -> p c n", c=DCH),
                    AF.Copy,
                )

            # ---- small constants (host pre-transposed; idle SWDGE queues) ----
            maskp = pp.tile([P, SC_CHUNKS], F32)
            nc.gpsimd.dma_start(maskp[:], maskf[:, :])
            bq_sb = pp.tile([P, PAIRS], F32)
            bk_sb = pp.tile([P, PAIRS], F32)
            nc.gpsimd.dma_start(bq_sb[:], bq[:, :])
            nc.gpsimd.dma_start(bk_sb[:], bk[:, :])
            bv_f32 = pp.tile([1, 256], F32)
            nc.gpsimd.dma_start(bv_f32[:], bv[None, :])
            bv_row = pp.tile([1, 256], BF16)
            nc.vector.tensor_copy(bv_row[:], bv_f32[:])
            ones_col = pp.tile([1, P], BF16)
            nc.gpsimd.memset(ones_col[:], 1.0)
            # tiny warmup exp so the ACT table load happens in the prefix
            expw = pp.tile([1, 8], F32)
            nc.scalar.activation(expw[:], bv_f32[:, 0:8], AF.Exp, scale=0.0)

            # weights: fp32 via SWDGE plain DMA + DVE cast (wv first), wo cast DMA
            def load_weight(dst, src, n):
                stg = ws.tile([P, DCH * 256], F32, tag="wstg", name=f"stg_{n}")
                nc.gpsimd.dma_start(
                    stg[:].rearrange("p (c n) -> p c n", c=DCH),
                    src.rearrange("(c p) n -> p c n", p=P),
                )
                nc.vector.tensor_copy(
                    dst[:].rearrange("p c n -> p (c n)"), stg[:]
                )

            load_weight(wv_sb, wv, "wv")
            load_weight(wq_sb, wq, "wq")
            load_weight(wk_sb, wk, "wk")
            nc.gpsimd.dma_start(wo_sb[:], wo.rearrange("(c p) n -> p c n", p=P))

            # mask columns of V_ext (disjoint from the V column writes)
            ve4 = VE[:].rearrange("p st (h c) -> p st h c", h=HPC)
            nc.vector.tensor_copy(
                ve4[:, :, :, DH : DH + 1],
                maskp[:, :, None, None].to_broadcast([P, SC_CHUNKS, HPC, 1]),
            )

            def v_proj(st):
                pv = ps_w.tile([P, QW], F32, tag="w")
                nc.tensor.matmul(
                    pv[:, :256], ones_col[:], bv_row[:], start=True, stop=False
                )
                for dc in range(DCH):
                    nc.tensor.matmul(
                        pv[:, :256],
                        xT[:, dc, st * P : (st + 1) * P],
                        wv_sb[:, dc, :],
                        start=False,
                        stop=(dc == DCH - 1),
                    )
                nc.scalar.activation(
                    ve4[:, st, :, 0:DH],
                    pv[:, :256].rearrange("p (h c) -> p h c", h=HPC),
                    AF.Copy,
                    scale=maskp[:, st : st + 1],
                )

            def proj_one(dst, w_sb, b_sb, pr, qt):
                sl = slice(qt * QW, (qt + 1) * QW)
                pq = ps_w.tile([P, QW], F32, tag="w")
                for dc in range(DCH):
                    nc.tensor.matmul(
                        pq[:],
                        w_sb[:, dc, pr * P : (pr + 1) * P],
                        xT[:, dc, sl],
                        start=(dc == 0),
                        stop=(dc == DCH - 1),
                    )
                nc.scalar.activation(
                    dst[:, pr, sl], pq[:], AF.Identity, bias=b_sb[:, pr : pr + 1]
                )

            def attention(pr, qt, fillers=None):
                qsl = slice(qt * QW, (qt + 1) * QW)
                cps = [
                    ps_ctx.tile([P, QW], F32, tag="ctx", name=f"ctx{hh}")
                    for hh in range(2)
                ]

                def emit_ctx(kc, et_a, et_d):
                    for hh, et in ((0, et_a), (1, et_d)):
                        h = 2 * pr + hh
                        nc.tensor.matmul(
                            cps[hh][: DH + 1, :],
                            VE[:, kc, h * (DH + 1) : (h + 1) * (DH + 1)],
                            et[:],
                            start=(kc == 0),
                            stop=(kc == SC_CHUNKS - 1),
                        )

                # Software-pipelined: ctx(kc) is emitted two slots behind
                # scores(kc) so the PE queue (strict FIFO in emission order)
                # never stalls waiting for exp(kc).
                pend = []
                for kc in range(SC_CHUNKS):
                    sc = ps_sc.tile([P, 2 * QW], F32, tag="sc")
                    for hh in range(2):
                        nc.tensor.matmul(
                            sc[:, hh * QW : (hh + 1) * QW],
                            KT[hh * DH : (hh + 1) * DH, pr, kc * P : (kc + 1) * P],
                            QT[hh * DH : (hh + 1) * DH, pr, qsl],
                            start=True,
                            stop=True,
                            tile_position=(hh * DH, 0),
                        )
                    # head 0 -> ACT exact exp; head 1 -> DVE Schraudolph.
                    # Separate tiles: a shared tile creates a false WAW dep
                    # between the two engines' writes.
                    et_a = ep.tile([P, QW], BF16, tag="eta", name="et_a")
                    nc.scalar.activation(
                        et_a[:], sc[:, 0:QW], AF.Exp, scale=float(SCALE)
                    )
                    et_d = ep.tile([P, QW], BF16, tag="etd", name="et_d")
                    nc.vector.tensor_scalar(
                        et_d.bitcast(I16)[:, :],
                        sc[:, QW : 2 * QW],
                        KA,
                        KB,
                        OP.mult,
                        OP.add,
                    )
                    pend.append((kc, et_a, et_d))
                    if len(pend) == 3:
                        emit_ctx(*pend.pop(0))
                for item in pend:
                    emit_ctx(*item)
                # normalize: den -> SBUF (ACT; reciprocal_approx_fast reads
                # the wrong PSUM rows on HW) -> 1/den (DVE) -> bcast -> mul
                for hh in range(2):
                    den = sp.tile([1, QW], F32, tag="den", name=f"den{hh}")
                    nc.vector.tensor_copy(den[:], cps[hh][DH : DH + 1, :])
                    rec = sp.tile([1, QW], F32, tag="rec", name=f"rec{hh}")
                    nc.vector.reciprocal_approx_fast(rec[:], den[:])
                    recB = sp.tile([DH, QW], F32, tag="recB", name=f"recB{hh}")
                    nc.gpsimd.partition_broadcast(recB[:], rec[:])
                    nc.vector.tensor_mul(
                        ctxT[hh * DH : (hh + 1) * DH, pr, qsl],
                        cps[hh][:DH, :],
                        recB[:],
                    )

            def out_proj(st):
                ob = op_.tile([P, D], BF16, tag="ob")
                for nt in range(2):
                    po = ps_w.tile([P, QW], F32, tag="w")
                    for pr in range(PAIRS):
                        nc.tensor.matmul(
                            po[:],
                            ctxT[:, pr, st * P : (st + 1) * P],
                            wo_sb[:, pr, nt * QW : (nt + 1) * QW],
                            start=(pr == 0),
                            stop=(pr == PAIRS - 1),
                        )
                    if nt == 0:
                        nc.scalar.activation(ob[:, 0:QW], po[:], AF.Copy)
                    else:
                        nc.vector.tensor_copy(ob[:, QW:D], po[:])
                nc.sync.dma_start(out[st * P : (st + 1) * P, :], ob[:])

            # ---- emission order (sets scheduling priority) ----
            # Attention needs ALL of KT/VE (keys span S) but only its own
            # QT tile, so: V and K projections up front, Q per-tile, and
            # the next pair's K interleaved into the current attention
            # phase — the exp engines (ACT/DVE) start as early as possible.
            # pipeline: transpose(st+1) ahead of v_proj(st) so the PE queue
            # doesn't stall on the ACT evacuation of xT(st)
            transpose_st(0)
            for st in range(1, ST_TILES):
                transpose_st(st)
                v_proj(st - 1)
            v_proj(ST_TILES - 1)
            for qt in range(QT_TILES):
                proj_one(KT, wk_sb, bk_sb, 0, qt)
            proj_one(QT, wq_sb, bq_sb, 0, 0)
            for qt in range(QT_TILES):
                attention(0, qt)
                if qt < QT_TILES - 1:
                    proj_one(QT, wq_sb, bq_sb, 0, qt + 1)
                proj_one(KT, wk_sb, bk_sb, 1, qt)
            proj_one(QT, wq_sb, bq_sb, 1, 0)
            for qt in range(QT_TILES):
                attention(1, qt)
                if qt < QT_TILES - 1:
                    proj_one(QT, wq_sb, bq_sb, 1, qt + 1)
                for st in range(4 * qt, 4 * qt + 4):
                    out_proj(st)

    nc.finalize()
    return nc


def shard_inputs(x, Wq, bq, Wk, bk, Wv, bv, Wo, bo, mask):
    """Full inputs -> list of 8 per-core input maps."""
    maskf = (~np.asarray(mask)).astype(np.float32)  # 1.0 = keep
    x_bf = np.asarray(x, dtype=np.float32).astype(ml_dtypes.bfloat16)
    ins = []
    for c in range(N_CORES):
        b, g = divmod(c, 4)
        cs = slice(g * 256, (g + 1) * 256)
        ins.append(
            {
                "x": np.ascontiguousarray(x_bf[b]),
                "wq": np.ascontiguousarray(Wq[:, cs]),
                "wk": np.ascontiguousarray(Wk[:, cs]),
                "wv": np.ascontiguousarray(Wv[:, cs]),
                "wo": np.ascontiguousarray(Wo[cs, :]),
                # biases/mask pre-transposed to the on-chip layouts
                "bq": np.ascontiguousarray(np.asarray(bq[cs]).reshape(2, P).T),
                "bk": np.ascontiguousarray(np.asarray(bk[cs]).reshape(2, P).T),
                "bv": np.ascontiguousarray(bv[cs]),
                "maskf": np.ascontiguousarray(
                    np.asarray(maskf[b]).reshape(SC_CHUNKS, P).T
                ),
            }
        )
    return ins


def gather_outputs(results, bo):
    """8 per-core partial outputs -> full (2, S, D) fp32 output."""
    outs = []
    for b in range(2):
        acc = results[4 * b]["out"].astype(np.float32)
        for g in range(1, 4):
            acc = acc + results[4 * b + g]["out"].astype(np.float32)
        outs.append(acc + np.asarray(bo, dtype=np.float32))
    return np.stack(outs, axis=0)


_NC_CACHE = []


def _get_nc():
    if not _NC_CACHE:
        _NC_CACHE.append(build())
    return _NC_CACHE[0]


def run_sharded(inputs, trace=False, tmpdir=None):
    """Shard, run on cores 0-7, gather. Returns (output, BassKernelResults)."""
    nc = _get_nc()
    ins = shard_inputs(**inputs)
    res = run_bass_kernel_spmd(
        nc, ins, core_ids=list(range(N_CORES)), trace=trace, tmpdir=tmpdir
    )
    full = gather_outputs(res.results, inputs["bo"])
    return full, res


def kernel(**inputs) -> np.ndarray:
    full, _ = run_sharded(inputs, trace=False)
    return full


# revision 28
# speedup vs baseline: 1.0159x; 1.0159x over previous
"""Multi-head attention (B=2, S=2048, D=1024, H=16, Dh=64) on 8 Trainium2
NeuronCores via Bass/Tile.

Sharding: data-parallel over the 2 batches x tensor-parallel over head
groups (16 heads -> 4 groups of 4). Core c = 4*b + g handles batch b and
heads 4g..4g+3 with the matching column/row slices of Wq/Wk/Wv/Wo. Each
core returns its partial output projection (bf16); the host sums the 4
partials per batch and adds bo.

v2 vs v1:
  - x is pre-cast to bf16 on host; x^T materialized via DMA XBAR
    transposes (DRAM -> SBUF) instead of PE transpose + DVE evacuation.
  - exp split across engines: ACT does the first AW columns of each
    score tile exactly; DVE does the rest with a Schraudolph-style
    int16 bit-trick (bits = KA*score + KB, bitcast as bf16 ~= exp).
    The softmax denominator uses the same approx values, so the
    common-mode error cancels; residual ~2% rms on half the heads.
  - V bias folded into the V-projection matmul via a K=1 ones-row MM;
    V/QK PSUM evacuations moved to the ACT engine (activation Copy /
    Identity with per-partition scale/bias).
  - weights loaded as fp32 via fast sync DMA and cast to bf16 on DVE
    (frees the slow SWDGE casting queues; x no longer needs them).
  - out partials stored bf16 (halves the output DMA).

Per-core kernel (4 heads = 2 "pairs" of 64-dim heads stacked to fill the
128-partition dim), bf16 matmul datapath with fp32 PSUM accumulation:
  xT   = dma_transpose(x_bf16)          [128 (d), 8, S]
  QT   = Wq_g^T x^T + bq_g              [128 (2 heads x 64), 2 pairs, S]
  KT   = Wk_g^T x^T + bk_g              (same layout)
  V_ext= [(x Wv_g + bv_g) * maskf | maskf]   [s, chunk, 4*(64+1)] bf16
  per pair, per q-tile (512 queries), per key chunk (128 keys):
    scT [128k, 2x512q] = KT_chunk^T @ QT_tile   (2 heads row-tiled, concurrent)
    eT  = exp(SCALE * scT)   (ACT exact on cols<AW, DVE Schraudolph on rest)
    ctx_h[65, 512] += V_ext_chunk^T @ eT_h      (row 64 = softmax denom)
  normalize: den -> 1/x (DVE) -> broadcast (GPSIMD) -> ctxT = ctx*rec (DVE)
  out_partial = ctxT^T @ Wo_g           (PSUM accum over the 2 pairs)
"""

import numpy as np
import ml_dtypes

import concourse.bacc as bacc
import concourse.mybir as mybir
import concourse.tile as tile
from concourse.bass_utils import run_bass_kernel_spmd
from concourse.masks import make_identity

F32 = mybir.dt.float32
BF16 = mybir.dt.bfloat16
I16 = mybir.dt.int16
AF = mybir.ActivationFunctionType
OP = mybir.AluOpType

S = 2048
D = 1024
HPC = 4                  # heads per core
DH = 64
PAIRS = 2                # head pairs per core
P = 128
SC_CHUNKS = S // P       # 16 key chunks
QT_TILES = 4             # q tiles of 512
QW = 512                 # q tile width
ST_TILES = S // P        # 16 s tiles
DCH = D // P             # 8 D chunks
SCALE = 1.0 / np.sqrt(DH)

# exp split: ACT handles et[:, 0:AW] exactly; DVE approximates the rest.
AW = 512
# Schraudolph constants for bf16: bits = KA*score + KB, int16-cast,
# bitcast bf16 ~= exp(SCALE*score). C=5.35 calibrated (maxrel ~3.3%).
KA = float(SCALE * 128.0 / np.log(2.0))
KB = float(16256.0 - 5.35)

N_CORES = 8


def build():
    nc = bacc.Bacc(None, target_bir_lowering=False, num_swdge_queues=4)

    x = nc.dram_tensor("x", [S, D], BF16, kind="ExternalInput")
    wq = nc.dram_tensor("wq", [D, 256], F32, kind="ExternalInput")
    wk = nc.dram_tensor("wk", [D, 256], F32, kind="ExternalInput")
    wv = nc.dram_tensor("wv", [D, 256], F32, kind="ExternalInput")
    wo = nc.dram_tensor("wo", [256, D], F32, kind="ExternalInput")
    bq = nc.dram_tensor("bq", [P, PAIRS], F32, kind="ExternalInput")
    bk = nc.dram_tensor("bk", [P, PAIRS], F32, kind="ExternalInput")
    bv = nc.dram_tensor("bv", [256], F32, kind="ExternalInput")
    maskf = nc.dram_tensor("maskf", [P, SC_CHUNKS], F32, kind="ExternalInput")
    out = nc.dram_tensor("out", [S, D], BF16, kind="ExternalOutput")

    with tile.TileContext(nc) as tc:
        with (
            tc.tile_pool(name="persist", bufs=1) as pp,
            tc.tile_pool(name="xstage", bufs=3) as xs,
            tc.tile_pool(name="wstage", bufs=2) as ws,
            tc.tile_pool(name="expp", bufs=5) as ep,
            tc.tile_pool(name="ostage", bufs=2) as op_,
            tc.tile_pool(name="smalls", bufs=3) as sp,
            tc.tile_pool(name="ps_sc", bufs=2, space="PSUM") as ps_sc,
            tc.tile_pool(name="ps_ctx", bufs=2, space="PSUM") as ps_ctx,
            tc.tile_pool(name="ps_w", bufs=2, space="PSUM") as ps_w,
        ):
            # ---- persistent big tensors ----
            xT = pp.tile([P, DCH, S], BF16)
            QT = pp.tile([P, PAIRS, S], BF16)
            KT = pp.tile([P, PAIRS, S], BF16)
            VE = pp.tile([P, SC_CHUNKS, HPC * (DH + 1)], BF16)
            ctxT = pp.tile([P, PAIRS, S], BF16)
            wq_sb = pp.tile([P, DCH, 256], BF16)
            wk_sb = pp.tile([P, DCH, 256], BF16)
            wv_sb = pp.tile([P, DCH, 256], BF16)
            wo_sb = pp.tile([P, PAIRS, D], BF16)

            # x^T via PE transpose-mode (DMA XBAR transpose is only ~60GB/s
            # and stalls the prefix). 8 transposes batch into one PSUM
            # region, evacuated by a single ACT copy per s-tile.
            ident = pp.tile([P, P], BF16)
            make_identity(nc, ident[:])

            def transpose_st(st):
                xst = xs.tile([P, D], BF16, tag="xst")
                nc.sync.dma_start(xst[:], x[st * P : (st + 1) * P, :])
                pt = ps_sc.tile([P, DCH * P], BF16, tag="sc")
                for dc in range(DCH):
                    nc.tensor.transpose(
                        pt[:, dc * P : (dc + 1) * P],
                        xst[:, dc * P : (dc + 1) * P],
                        ident[:],
                    )
                nc.scalar.activation(
                    xT[:, :, st * P : (st + 1) * P],
                    pt[:].rearrange("p (c n) -> p c n", c=DCH),
                    AF.Copy,
                )

            # ---- small constants (host pre-transposed; idle SWDGE queues) ----
            maskp = pp.tile([P, SC_CHUNKS], F32)
            nc.gpsimd.dma_start(maskp[:], maskf[:, :])
            bq_sb = pp.tile([P, PAIRS], F32)
            bk_sb = pp.tile([P, PAIRS], F32)
            nc.gpsimd.dma_start(bq_sb[:], bq[:, :])
            nc.gpsimd.dma_start(bk_sb[:], bk[:, :])
            bv_f32 = pp.tile([1, 256], F32)
            nc.gpsimd.dma_start(bv_f32[:], bv[None, :])
            bv_row = pp.tile([1, 256], BF16)
            nc.vector.tensor_copy(bv_row[:], bv_f32[:])
            ones_col = pp.tile([1, P], BF16)
            nc.gpsimd.memset(ones_col[:], 1.0)
            # tiny warmup exp so the ACT table load happens in the prefix
            expw = pp.tile([1, 8], F32)
            nc.scalar.activation(expw[:], bv_f32[:, 0:8], AF.Exp, scale=0.0)

            # weights: fp32 via SWDGE plain DMA + DVE cast (wv first), wo cast DMA
            def load_weight(dst, src, n):
                stg = ws.tile([P, DCH * 256], F32, tag="wstg", name=f"stg_{n}")
                nc.gpsimd.dma_start(
                    stg[:].rearrange("p (c n) -> p c n", c=DCH),
                    src.rearrange("(c p) n -> p c n", p=P),
                )
                nc.vector.tensor_copy(
                    dst[:].rearrange("p c n -> p (c n)"), stg[:]
                )

            load_weight(wv_sb, wv, "wv")
            load_weight(wq_sb, wq, "wq")
            load_weight(wk_sb, wk, "wk")
            nc.gpsimd.dma_start(wo_sb[:], wo.rearrange("(c p) n -> p c n", p=P))

            # mask columns of V_ext (disjoint from the V column writes)
            ve4 = VE[:].rearrange("p st (h c) -> p st h c", h=HPC)
            nc.vector.tensor_copy(
                ve4[:, :, :, DH : DH + 1],
                maskp[:, :, None, None].to_broadcast([P, SC_CHUNKS, HPC, 1]),
            )

            def v_proj(st):
                pv = ps_w.tile([P, QW], F32, tag="w")
                nc.tensor.matmul(
                    pv[:, :256], ones_col[:], bv_row[:], start=True, stop=False
                )
                for dc in range(DCH):
                    nc.tensor.matmul(
                        pv[:, :256],
                        xT[:, dc, st * P : (st + 1) * P],
                        wv_sb[:, dc, :],
                        start=False,
                        stop=(dc == DCH - 1),
                    )
                nc.scalar.activation(
                    ve4[:, st, :, 0:DH],
                    pv[:, :256].rearrange("p (h c) -> p h c", h=HPC),
                    AF.Copy,
                    scale=maskp[:, st : st + 1],
                )

            def proj_one(dst, w_sb, b_sb, pr, qt):
                sl = slice(qt * QW, (qt + 1) * QW)
                pq = ps_w.tile([P, QW], F32, tag="w")
                for dc in range(DCH):
                    nc.tensor.matmul(
                        pq[:],
                        w_sb[:, dc, pr * P : (pr + 1) * P],
                        xT[:, dc, sl],
                        start=(dc == 0),
                        stop=(dc == DCH - 1),
                    )
                nc.scalar.activation(
                    dst[:, pr, sl], pq[:], AF.Identity, bias=b_sb[:, pr : pr + 1]
                )

            def attention(pr, qt, fillers=None):
                qsl = slice(qt * QW, (qt + 1) * QW)
                cps = [
                    ps_ctx.tile([P, QW], F32, tag="ctx", name=f"ctx{hh}")
                    for hh in range(2)
                ]

                def emit_ctx(kc, et_a, et_d):
                    for hh, et in ((0, et_a), (1, et_d)):
                        h = 2 * pr + hh
                        nc.tensor.matmul(
                            cps[hh][: DH + 1, :],
                            VE[:, kc, h * (DH + 1) : (h + 1) * (DH + 1)],
                            et[:],
                            start=(kc == 0),
                            stop=(kc == SC_CHUNKS - 1),
                        )

                # Software-pipelined: ctx(kc) is emitted two slots behind
                # scores(kc) so the PE queue (strict FIFO in emission order)
                # never stalls waiting for exp(kc).
                pend = []
                for kc in range(SC_CHUNKS):
                    sc = ps_sc.tile([P, 2 * QW], F32, tag="sc")
                    for hh in range(2):
                        nc.tensor.matmul(
                            sc[:, hh * QW : (hh + 1) * QW],
                            KT[hh * DH : (hh + 1) * DH, pr, kc * P : (kc + 1) * P],
                            QT[hh * DH : (hh + 1) * DH, pr, qsl],
                            start=True,
                            stop=True,
                            tile_position=(hh * DH, 0),
                        )
                    # head 0 -> ACT exact exp; head 1 -> DVE Schraudolph.
                    # Separate tiles: a shared tile creates a false WAW dep
                    # between the two engines' writes.
                    et_a = ep.tile([P, QW], BF16, tag="eta", name="et_a")
                    nc.scalar.activation(
                        et_a[:], sc[:, 0:QW], AF.Exp, scale=float(SCALE)
                    )
                    et_d = ep.tile([P, QW], BF16, tag="etd", name="et_d")
                    nc.vector.tensor_scalar(
                        et_d.bitcast(I16)[:, :],
                        sc[:, QW : 2 * QW],
                        KA,
                        KB,
                        OP.mult,
                        OP.add,
                    )
                    pend.append((kc, et_a, et_d))
                    if len(pend) == 3:
                        emit_ctx(*pend.pop(0))
                for item in pend:
                    emit_ctx(*item)
                # normalize: den -> SBUF (ACT; reciprocal_approx_fast reads
                # the wrong PSUM rows on HW) -> 1/den (DVE) -> bcast -> mul
                for hh in range(2):
                    den = sp.tile([1, QW], F32, tag="den", name=f"den{hh}")
                    nc.scalar.activation(den[:], cps[hh][DH : DH + 1, :], AF.Copy)
                    rec = sp.tile([1, QW], F32, tag="rec", name=f"rec{hh}")
                    nc.vector.reciprocal_approx_fast(rec[:], den[:])
                    recB = sp.tile([DH, QW], F32, tag="recB", name=f"recB{hh}")
                    nc.gpsimd.partition_broadcast(recB[:], rec[:])
                    nc.vector.tensor_mul(
                        ctxT[hh * DH : (hh + 1) * DH, pr, qsl],
                        cps[hh][:DH, :],
                        recB[:],
                    )

            def out_proj(st):
                ob = op_.tile([P, D], BF16, tag="ob")
                for nt in range(2):
                    po = ps_w.tile([P, QW], F32, tag="w")
                    for pr in range(PAIRS):
                        nc.tensor.matmul(
                            po[:],
                            ctxT[:, pr, st * P : (st + 1) * P],
                            wo_sb[:, pr, nt * QW : (nt + 1) * QW],
                            start=(pr == 0),
                            stop=(pr == PAIRS - 1),
                        )
                    if nt == 0:
                        nc.scalar.activation(ob[:, 0:QW], po[:], AF.Copy)
                    else:
                        nc.vector.tensor_copy(ob[:, QW:D], po[:])
                nc.sync.dma_start(out[st * P : (st + 1) * P, :], ob[:])

            # ---- emission order (sets scheduling priority) ----
            # Attention needs ALL of KT/VE (keys span S) but only its own
            # QT tile, so: V and K projections up front, Q per-tile, and
            # the next pair's K interleaved into the current attention
            # phase — the exp engines (ACT/DVE) start as early as possible.
            # pipeline: transpose(st+1) ahead of v_proj(st) so the PE queue
            # doesn't stall on the ACT evacuation of xT(st)
            transpose_st(0)
            for st in range(1, ST_TILES):
                transpose_st(st)
                v_proj(st - 1)
            v_proj(ST_TILES - 1)
            for qt in range(QT_TILES):
                proj_one(KT, wk_sb, bk_sb, 0, qt)
            proj_one(QT, wq_sb, bq_sb, 0, 0)
            for qt in range(QT_TILES):
                attention(0, qt)
                if qt < QT_TILES - 1:
                    proj_one(QT, wq_sb, bq_sb, 0, qt + 1)
                proj_one(KT, wk_sb, bk_sb, 1, qt)
            proj_one(QT, wq_sb, bq_sb, 1, 0)
            for qt in range(QT_TILES):
                attention(1, qt)
                if qt < QT_TILES - 1:
                    proj_one(QT, wq_sb, bq_sb, 1, qt + 1)
                for st in range(4 * qt, 4 * qt + 4):
                    out_proj(st)

    nc.finalize()
    return nc


def shard_inputs(x, Wq, bq, Wk, bk, Wv, bv, Wo, bo, mask):
    """Full inputs -> list of 8 per-core input maps."""
    maskf = (~np.asarray(mask)).astype(np.float32)  # 1.0 = keep
    x_bf = np.asarray(x, dtype=np.float32).astype(ml_dtypes.bfloat16)
    ins = []
    for c in range(N_CORES):
        b, g = divmod(c, 4)
        cs = slice(g * 256, (g + 1) * 256)
        ins.append(
            {
                "x": np.ascontiguousarray(x_bf[b]),
                "wq": np.ascontiguousarray(Wq[:, cs]),
                "wk": np.ascontiguousarray(Wk[:, cs]),
                "wv": np.ascontiguousarray(Wv[:, cs]),
                "wo": np.ascontiguousarray(Wo[cs, :]),
                # biases/mask pre-transposed to the on-chip layouts
                "bq": np.ascontiguousarray(np.asarray(bq[cs]).reshape(2, P).T),
                "bk": np.ascontiguousarray(np.asarray(bk[cs]).reshape(2, P).T),
                "bv": np.ascontiguousarray(bv[cs]),
                "maskf": np.ascontiguousarray(
                    np.asarray(maskf[b]).reshape(SC_CHUNKS, P).T
                ),
            }
        )
    return ins


def gather_outputs(results, bo):
    """8 per-core partial outputs -> full (2, S, D) fp32 output."""
    outs = []
    for b in range(2):
        acc = results[4 * b]["out"].astype(np.float32)
        for g in range(1, 4):
            acc = acc + results[4 * b + g]["out"].astype(np.float32)
        outs.append(acc + np.asarray(bo, dtype=np.float32))
    return np.stack(outs, axis=0)


_NC_CACHE = []


def _get_nc():
    if not _NC_CACHE:
        _NC_CACHE.append(build())
    return _NC_CACHE[0]


def run_sharded(inputs, trace=False, tmpdir=None):
    """Shard, run on cores 0-7, gather. Returns (output, BassKernelResults)."""
    nc = _get_nc()
    ins = shard_inputs(**inputs)
    res = run_bass_kernel_spmd(
        nc, ins, core_ids=list(range(N_CORES)), trace=trace, tmpdir=tmpdir
    )
    full = gather_outputs(res.results, inputs["bo"])
    return full, res


def kernel(**inputs) -> np.ndarray:
    full, _ = run_sharded(inputs, trace=False)
    return full
